# revision 15
# baseline (speedup 1.0000x reference)
"""GCN layer (4-relation message passing) on 8 Trainium2 NeuronCores.

out = sum_r (A_r @ inp) @ W_r + sum_r b_r,  A_r in COO form (dst, src, val).

Sharding: edges sharded by dst range; core c owns dst in [c*12500, (c+1)*12500).
Edges are bucketed per (dst-window of 64, relation) cell and padded to
128-edge blocks. Two block formats (pure placement of input values on the
host - no host arithmetic):

  sel-format (128 bf16 cols): [ msg 64 | val*onehot(dstloc) 64 ]
  gen-format ( 64 bf16 cols): [ msg 64 ] + f32 sideband [ dstloc | val ]

For gen-format blocks the device builds the val-weighted selection matrix
with one fused DVE op:  oh[p, j] = (iota[j] == dstloc[p]) * val[p].
The format mix (SEL_EVERY) balances HBM traffic against DVE throughput.

Per cell, PE accumulates  aggT_wr [64f, 64n] += MSG_b^T @ SEL_b  in PSUM
(edge_val scaling and the dst segment-sum happen inside this matmul).
Phase 2 per window (relation-outer so the stationary W_r is reused):
  outT_w [64o, 64n] += W_r^T-contraction matmul(lhsT=W_r, rhs=aggT_wr)
plus bias via matmul(lhsT=bias[4, 64o], rhs=ones[4, 64n]).
"""

import math
from contextlib import ExitStack

import numpy as np

import concourse.bass as bass
import concourse.tile as tile
from concourse import bacc, mybir
from concourse.bass_utils import run_bass_kernel_spmd

# problem constants
N_NODES = 100000
N_REL = 4
N_EDGES = 1600000
IN_SIZE = 64
OUT_SIZE = 64

N_CORES = 8
NPC = N_NODES // N_CORES  # nodes (dst) per core
P = 128                   # partitions / edges per block
W = 64                    # dst-window width (nodes per psum tile)
SELW = P                  # sel-format block width in bf16 cols
GENW = IN_SIZE            # gen-format block width in bf16 cols
VW = 2                    # f32 sideband cols per gen block (dstloc, val)
GW = 4                    # windows per slab DMA group
SEL_EVERY = 4             # every n-th cell uses sel-format (rest gen-format)

F32 = mybir.dt.float32
BF16 = mybir.dt.bfloat16


def _np_bf16():
    import ml_dtypes
    return ml_dtypes.bfloat16


def _host_prep(inp, src, dst, edge_val):
    """Bucket/pad edges per (core, window, rel); build the two block slabs."""
    n_win = math.ceil(NPC / W)
    ncell = n_win * N_REL
    srcf = src.reshape(-1).astype(np.int64)
    dstf = dst.reshape(-1).astype(np.int64)
    valf = edge_val.reshape(-1).astype(np.float32)
    rel = np.repeat(np.arange(N_REL, dtype=np.int64), src.shape[1])

    core = dstf // NPC
    dloc = dstf % NPC
    win = dloc // W
    wloc = dloc % W
    cell = win * N_REL + rel
    key = core * ncell + cell

    counts = np.bincount(key, minlength=N_CORES * ncell).reshape(
        N_CORES, ncell)
    B = np.maximum((counts.max(axis=0) + P - 1) // P, 1).astype(np.int64)

    is_sel = (np.arange(ncell) % SEL_EVERY) == 0
    # block-start offsets within each format's region (block units)
    starts_s = np.zeros(ncell + 1, dtype=np.int64)
    np.cumsum(np.where(is_sel, B, 0), out=starts_s[1:])
    starts_g = np.zeros(ncell + 1, dtype=np.int64)
    np.cumsum(np.where(is_sel, 0, B), out=starts_g[1:])
    TS = int(starts_s[-1])
    TG = int(starts_g[-1])

    edt = _np_bf16()
    slab_s = np.zeros((N_CORES, P, max(TS, 1), SELW), dtype=edt)
    slab_g = np.zeros((N_CORES, P, max(TG, 1), GENW), dtype=edt)
    slab_v = np.zeros((N_CORES, P, max(TG, 1), VW), dtype=np.float32)

    order = np.argsort(key, kind="stable")
    grp_start = np.zeros(N_CORES * ncell, dtype=np.int64)
    np.cumsum(counts.reshape(-1)[:-1], out=grp_start[1:])
    j = np.arange(len(order), dtype=np.int64) - grp_start[key[order]]
    cell_o = cell[order]
    sel_o = is_sel[cell_o]
    p_row = j % P
    c_ord = core[order]
    msg = inp[srcf[order]].astype(edt)

    m = sel_o
    t_col = starts_s[cell_o[m]] + (j[m] // P)
    slab_s[c_ord[m], p_row[m], t_col, :IN_SIZE] = msg[m]
    slab_s[c_ord[m], p_row[m], t_col, IN_SIZE + wloc[order][m]] = (
        valf[order][m].astype(edt))

    m = ~sel_o
    t_col = starts_g[cell_o[m]] + (j[m] // P)
    slab_g[c_ord[m], p_row[m], t_col, :IN_SIZE] = msg[m]
    slab_v[c_ord[m], p_row[m], t_col, 0] = wloc[order][m].astype(np.float32)
    slab_v[c_ord[m], p_row[m], t_col, 1] = valf[order][m]

    return (n_win, B, is_sel, starts_s, starts_g, TS, TG,
            slab_s, slab_g, slab_v)


_PROG_CACHE = {}


def _build_program(n_win, B, is_sel, starts_s, starts_g, TS, TG):
    key = (W, GW, SEL_EVERY, tuple(int(b) for b in B))
    if key in _PROG_CACHE:
        return _PROG_CACHE[key]

    nc = bacc.Bacc("TRN2", target_bir_lowering=False, debug=False,
                   num_devices=N_CORES)
    wcat = nc.dram_tensor("wcat", [IN_SIZE, N_REL * OUT_SIZE], BF16,
                          kind="ExternalInput").ap()
    biasc = nc.dram_tensor("biasc", [N_REL, OUT_SIZE], BF16,
                           kind="ExternalInput").ap()
    iotac = nc.dram_tensor("iotac", [P, W], BF16,
                           kind="ExternalInput").ap()
    eslab_s = nc.dram_tensor("eslab_s", [P, max(TS, 1) * SELW], BF16,
                             kind="ExternalInput").ap()
    eslab_g = nc.dram_tensor("eslab_g", [P, max(TG, 1) * GENW], BF16,
                             kind="ExternalInput").ap()
    eslab_v = nc.dram_tensor("eslab_v", [P, max(TG, 1) * VW], F32,
                             kind="ExternalInput").ap()
    out = nc.dram_tensor("out", [W, n_win * OUT_SIZE], F32,
                         kind="ExternalOutput").ap()

    with tile.TileContext(nc) as tc, ExitStack() as ctx:
        p_const = ctx.enter_context(tc.tile_pool(name="p_const", bufs=1))
        p_ms = ctx.enter_context(tc.tile_pool(name="p_ms", bufs=2))
        p_mg = ctx.enter_context(tc.tile_pool(name="p_mg", bufs=2))
        p_oh = ctx.enter_context(tc.tile_pool(name="p_oh", bufs=6))
        p_agg = ctx.enter_context(tc.tile_pool(name="p_agg", bufs=2 * GW * N_REL + 2))
        p_out = ctx.enter_context(tc.tile_pool(name="p_out", bufs=1))
        ps_agg = ctx.enter_context(tc.tile_pool(name="ps_agg", bufs=3,
                                                space="PSUM"))
        ps_out = ctx.enter_context(tc.tile_pool(name="ps_out", bufs=GW,
                                                space="PSUM"))

        wt = p_const.tile([IN_SIZE, N_REL * OUT_SIZE], BF16)
        nc.sync.dma_start(wt[:], wcat[:])
        bt = p_const.tile([N_REL, OUT_SIZE], BF16)
        nc.sync.dma_start(bt[:], biasc[:])
        iot = p_const.tile([P, W], BF16)
        nc.sync.dma_start(iot[:], iotac[:])
        ones4 = p_const.tile([N_REL, W], BF16)
        nc.vector.memset(ones4[:], 1.0)
        outsb = p_out.tile([W, n_win * OUT_SIZE], F32)

        # max group extents (block units) for tile sizing
        def grp_rng(starts, w0, w1):
            return int(starts[w0 * N_REL]), int(starts[min(w1, n_win) * N_REL])

        bs_max = max(grp_rng(starts_s, w0, w0 + GW)[1]
                     - grp_rng(starts_s, w0, w0 + GW)[0]
                     for w0 in range(0, n_win, GW))
        bg_max = max(grp_rng(starts_g, w0, w0 + GW)[1]
                     - grp_rng(starts_g, w0, w0 + GW)[0]
                     for w0 in range(0, n_win, GW))

        for w0 in range(0, n_win, GW):
            w1 = min(w0 + GW, n_win)
            s0, s1 = grp_rng(starts_s, w0, w1)
            g0, g1 = grp_rng(starts_g, w0, w1)
            mts = p_ms.tile([P, max(bs_max, 1) * SELW], BF16, tag="ms")
            if s1 > s0:
                nc.sync.dma_start(mts[:, :(s1 - s0) * SELW],
                                  eslab_s[:, s0 * SELW:s1 * SELW])
            mtg = p_mg.tile([P, max(bg_max, 1) * GENW], BF16, tag="mg")
            mtv = p_mg.tile([P, max(bg_max, 1) * VW], F32, tag="mv")
            if g1 > g0:
                nc.sync.dma_start(mtg[:, :(g1 - g0) * GENW],
                                  eslab_g[:, g0 * GENW:g1 * GENW])
                nc.sync.dma_start(mtv[:, :(g1 - g0) * VW],
                                  eslab_v[:, g0 * VW:g1 * VW])

            aggs = {}
            for w in range(w0, w1):
                for r in range(N_REL):
                    c2 = w * N_REL + r
                    ps = ps_agg.tile([IN_SIZE, W], F32)
                    if is_sel[c2]:
                        b0 = int(starts_s[c2]) - s0
                        b1 = int(starts_s[c2 + 1]) - s0
                        for b in range(b0, b1):
                            o = b * SELW
                            nc.tensor.matmul(
                                out=ps[:],
                                lhsT=mts[:, o:o + IN_SIZE],
                                rhs=mts[:, o + IN_SIZE:o + SELW],
                                start=(b == b0), stop=(b == b1 - 1))
                    else:
                        b0 = int(starts_g[c2]) - g0
                        b1 = int(starts_g[c2 + 1]) - g0
                        for b in range(b0, b1):
                            o = b * GENW
                            oh = p_oh.tile([P, W], BF16, tag="oh")
                            nc.vector.tensor_scalar(
                                out=oh[:],
                                in0=iot[:],
                                scalar1=mtv[:, b * VW:b * VW + 1],
                                scalar2=mtv[:, b * VW + 1:b * VW + 2],
                                op0=mybir.AluOpType.is_equal,
                                op1=mybir.AluOpType.mult)
                            nc.tensor.matmul(
                                out=ps[:],
                                lhsT=mtg[:, o:o + IN_SIZE],
                                rhs=oh[:],
                                start=(b == b0), stop=(b == b1 - 1))
                    agg = p_agg.tile([IN_SIZE, W], BF16, name="agg",
                                     tag="agg")
                    nc.scalar.copy(agg[:], ps[:])
                    aggs[(w, r)] = agg

            pos = {}
            for w in range(w0, w1):
                pos[w] = ps_out.tile([OUT_SIZE, W], F32, name="pos",
                                     tag="pos")
            for r in range(N_REL):
                for w in range(w0, w1):
                    nc.tensor.matmul(
                        out=pos[w][:],
                        lhsT=wt[:, r * OUT_SIZE:(r + 1) * OUT_SIZE],
                        rhs=aggs[(w, r)][:],
                        start=(r == 0), stop=False)
            for w in range(w0, w1):
                nc.tensor.matmul(
                    out=pos[w][:],
                    lhsT=bt[:],
                    rhs=ones4[:],
                    start=False, stop=True)
                nc.scalar.copy(
                    outsb[:, w * OUT_SIZE:(w + 1) * OUT_SIZE], pos[w][:])

        nc.sync.dma_start(out[:], outsb[:])

    nc.compile()
    _PROG_CACHE[key] = nc
    return nc


def _make_in_maps(inp, src, dst, edge_val, weights, bias):
    inp = np.asarray(inp, dtype=np.float32)
    src = np.asarray(src)
    dst = np.asarray(dst)
    edge_val = np.asarray(edge_val, dtype=np.float32)
    weights = np.asarray(weights, dtype=np.float32)
    bias = np.asarray(bias, dtype=np.float32)

    (n_win, B, is_sel, starts_s, starts_g, TS, TG,
     slab_s, slab_g, slab_v) = _host_prep(inp, src, dst, edge_val)
    nc = _build_program(n_win, B, is_sel, starts_s, starts_g, TS, TG)

    edt = _np_bf16()
    wcat = np.ascontiguousarray(
        weights.transpose(1, 0, 2).reshape(IN_SIZE, N_REL * OUT_SIZE)
    ).astype(edt)
    biasc = bias.astype(edt)
    iotac = np.broadcast_to(
        np.arange(W, dtype=np.float32), (P, W)).astype(edt)

    in_maps = []
    for c in range(N_CORES):
        in_maps.append({
            "wcat": wcat,
            "biasc": biasc,
            "iotac": np.ascontiguousarray(iotac),
            "eslab_s": slab_s[c].reshape(P, -1),
            "eslab_g": slab_g[c].reshape(P, -1),
            "eslab_v": slab_v[c].reshape(P, -1),
        })
    return nc, in_maps, n_win


def _unshard(res, n_win):
    parts = []
    for c in range(N_CORES):
        arr = res.results[c]["out"].reshape(OUT_SIZE, n_win, W)
        nodes = arr.transpose(1, 2, 0).reshape(n_win * W, OUT_SIZE)
        parts.append(nodes[:NPC])
    return np.concatenate(parts, axis=0).astype(np.float32)


def kernel(inp, src, dst, edge_val, weights, bias):
    nc, in_maps, n_win = _make_in_maps(inp, src, dst, edge_val, weights, bias)
    res = run_bass_kernel_spmd(nc, in_maps, list(range(N_CORES)))
    return _unshard(res, n_win)


# revision 16
# speedup vs baseline: 1.3134x; 1.3134x over previous
"""GCN layer (4-relation message passing) on 8 Trainium2 NeuronCores.

out = sum_r (A_r @ inp) @ W_r + sum_r b_r,  A_r in COO form (dst, src, val).

Sharding: edges sharded by dst range; core c owns dst in [c*12500, (c+1)*12500).
Edges are bucketed per (dst-window of W nodes, relation) cell and padded to
128-edge blocks. Host stages, per block, a [128, 64+W] bf16 slab: cols 0:64
hold the message rows inp[src], cols 64:64+W hold the selection matrix
S[p, j] = val_p * (j == dstloc_p)  (pure placement of input values - no
host arithmetic). Device does all FLOPs:

per (window w, relation r) cell, per block b:
  PE accumulates  aggT_wr [64f, W] += MSG_b^T @ S_b   in PSUM
  (edge_val scaling and the dst segment-sum happen inside this matmul).
Cells alternate between PSUM partition halves 0:64 / 64:128 so consecutive
cells' matmuls land in different PE column groups (weight-load overlap).
Phase 2 per window, relation-outer so the stationary W_r is loaded once per
group:  outT_w [64o, W] += matmul(lhsT=W_r[64f, 64o], rhs=aggT_wr)
plus bias via matmul(lhsT=bias[4, 64o], rhs=ones[4, W]).  PSUM->SBUF copies
alternate between the Scalar and Vector engines.
"""

import math
from contextlib import ExitStack

import numpy as np

import concourse.bass as bass
import concourse.tile as tile
from concourse import bacc, mybir
from concourse.bass_utils import run_bass_kernel_spmd

# problem constants
N_NODES = 100000
N_REL = 4
N_EDGES = 1600000
IN_SIZE = 64
OUT_SIZE = 64

N_CORES = 8
NPC = N_NODES // N_CORES  # nodes (dst) per core
P = 128                   # partitions / edges per block
W = 32                    # dst-window width (nodes per psum tile)
BW = IN_SIZE + W          # block slab width (msg cols + selection cols)
GW = 4                    # windows per slab DMA group

F32 = mybir.dt.float32
BF16 = mybir.dt.bfloat16


def _np_bf16():
    import ml_dtypes
    return ml_dtypes.bfloat16


def _host_prep(inp, src, dst, edge_val):
    """Bucket/pad edges per (core, window, rel); build block slabs."""
    n_win = math.ceil(NPC / W)
    ncell = n_win * N_REL
    srcf = src.reshape(-1).astype(np.int64)
    dstf = dst.reshape(-1).astype(np.int64)
    valf = edge_val.reshape(-1).astype(np.float32)
    rel = np.repeat(np.arange(N_REL, dtype=np.int64), src.shape[1])

    core = dstf // NPC
    dloc = dstf % NPC
    win = dloc // W
    wloc = dloc % W
    cell = win * N_REL + rel
    key = core * ncell + cell

    counts = np.bincount(key, minlength=N_CORES * ncell).reshape(
        N_CORES, ncell)
    B = np.maximum((counts.max(axis=0) + P - 1) // P, 1).astype(np.int64)
    starts = np.zeros(ncell + 1, dtype=np.int64)
    np.cumsum(B, out=starts[1:])
    T = int(starts[-1])

    edt = _np_bf16()
    slab = np.zeros((N_CORES, P, T, BW), dtype=edt)

    order = np.argsort(key, kind="stable")
    grp_start = np.zeros(N_CORES * ncell, dtype=np.int64)
    np.cumsum(counts.reshape(-1)[:-1], out=grp_start[1:])
    j = np.arange(len(order), dtype=np.int64) - grp_start[key[order]]
    t_col = starts[cell[order]] + (j // P)
    p_row = j % P
    c_ord = core[order]
    slab[c_ord, p_row, t_col, :IN_SIZE] = inp[srcf[order]].astype(edt)
    slab[c_ord, p_row, t_col, IN_SIZE + wloc[order]] = (
        valf[order].astype(edt))

    return n_win, B, starts, T, slab


_PROG_CACHE = {}


def _build_program(n_win, starts, T):
    key = (W, GW, tuple(int(s) for s in starts))
    if key in _PROG_CACHE:
        return _PROG_CACHE[key]

    nc = bacc.Bacc("TRN2", target_bir_lowering=False, debug=False,
                   num_devices=N_CORES)
    wcat = nc.dram_tensor("wcat", [IN_SIZE, N_REL * OUT_SIZE], BF16,
                          kind="ExternalInput").ap()
    biasc = nc.dram_tensor("biasc", [N_REL, OUT_SIZE], BF16,
                           kind="ExternalInput").ap()
    eslab = nc.dram_tensor("eslab", [P, T * BW], BF16,
                           kind="ExternalInput").ap()
    out = nc.dram_tensor("out", [OUT_SIZE, n_win * W], F32,
                         kind="ExternalOutput").ap()

    with tile.TileContext(nc) as tc, ExitStack() as ctx:
        p_const = ctx.enter_context(tc.tile_pool(name="p_const", bufs=1))
        p_msg = ctx.enter_context(tc.tile_pool(name="p_msg", bufs=2))
        p_agg = ctx.enter_context(
            tc.tile_pool(name="p_agg", bufs=2 * GW * N_REL + 2))
        p_out = ctx.enter_context(tc.tile_pool(name="p_out", bufs=1))
        ps_agg = ctx.enter_context(tc.tile_pool(name="ps_agg", bufs=3,
                                                space="PSUM"))
        ps_out = ctx.enter_context(tc.tile_pool(name="ps_out", bufs=GW,
                                                space="PSUM"))

        wt = p_const.tile([IN_SIZE, N_REL * OUT_SIZE], BF16)
        nc.sync.dma_start(wt[:], wcat[:])
        bt = p_const.tile([N_REL, OUT_SIZE], BF16)
        nc.sync.dma_start(bt[:], biasc[:])
        ones4 = p_const.tile([N_REL, W], BF16)
        nc.vector.memset(ones4[:], 1.0)
        outsb = p_out.tile([OUT_SIZE, n_win * W], F32)

        bg_max = max(
            int(starts[min(w0 + GW, n_win) * N_REL] - starts[w0 * N_REL])
            for w0 in range(0, n_win, GW))

        ncopy = 0
        for w0 in range(0, n_win, GW):
            w1 = min(w0 + GW, n_win)
            t0, t1 = int(starts[w0 * N_REL]), int(starts[w1 * N_REL])
            bg = t1 - t0
            mt = p_msg.tile([P, bg_max * BW], BF16, tag="msg")
            nc.sync.dma_start(mt[:, :bg * BW], eslab[:, t0 * BW:t1 * BW])

            aggs = {}
            for w in range(w0, w1):
                for r in range(N_REL):
                    c2 = w * N_REL + r
                    b0, b1 = int(starts[c2]) - t0, int(starts[c2 + 1]) - t0
                    ps = ps_agg.tile([P, W], F32, name="ps", tag="ps")
                    h = (ncopy % 2) * IN_SIZE
                    psh = ps[h:h + IN_SIZE, :]
                    for b in range(b0, b1):
                        o = b * BW
                        nc.tensor.matmul(
                            out=psh,
                            lhsT=mt[:, o:o + IN_SIZE],
                            rhs=mt[:, o + IN_SIZE:o + BW],
                            start=(b == b0), stop=(b == b1 - 1))
                    agg = p_agg.tile([IN_SIZE, W], BF16, name="agg",
                                     tag="agg")
                    if ncopy % 2 == 0:
                        nc.scalar.copy(agg[:], psh)
                    else:
                        nc.vector.tensor_copy(agg[:], psh)
                    ncopy += 1
                    aggs[(w, r)] = agg

            pos = {}
            for w in range(w0, w1):
                pos[w] = ps_out.tile([OUT_SIZE, W], F32, name="pos",
                                     tag="pos")
            for r in range(N_REL):
                for w in range(w0, w1):
                    nc.tensor.matmul(
                        out=pos[w][:],
                        lhsT=wt[:, r * OUT_SIZE:(r + 1) * OUT_SIZE],
                        rhs=aggs[(w, r)][:],
                        start=(r == 0), stop=False)
            for w in range(w0, w1):
                nc.tensor.matmul(
                    out=pos[w][:],
                    lhsT=bt[:],
                    rhs=ones4[:],
                    start=False, stop=True)
                if w % 2 == 0:
                    nc.scalar.copy(outsb[:, w * W:(w + 1) * W], pos[w][:])
                else:
                    nc.vector.tensor_copy(
                        outsb[:, w * W:(w + 1) * W], pos[w][:])

        nc.sync.dma_start(out[:], outsb[:])

    nc.compile()
    _PROG_CACHE[key] = nc
    return nc


def _make_in_maps(inp, src, dst, edge_val, weights, bias):
    inp = np.asarray(inp, dtype=np.float32)
    src = np.asarray(src)
    dst = np.asarray(dst)
    edge_val = np.asarray(edge_val, dtype=np.float32)
    weights = np.asarray(weights, dtype=np.float32)
    bias = np.asarray(bias, dtype=np.float32)

    n_win, B, starts, T, slab = _host_prep(inp, src, dst, edge_val)
    nc = _build_program(n_win, starts, T)

    edt = _np_bf16()
    wcat = np.ascontiguousarray(
        weights.transpose(1, 0, 2).reshape(IN_SIZE, N_REL * OUT_SIZE)
    ).astype(edt)
    biasc = bias.astype(edt)

    in_maps = []
    for c in range(N_CORES):
        in_maps.append({
            "wcat": wcat,
            "biasc": biasc,
            "eslab": slab[c].reshape(P, T * BW),
        })
    return nc, in_maps, n_win


def _unshard(res, n_win):
    parts = []
    for c in range(N_CORES):
        arr = res.results[c]["out"].reshape(OUT_SIZE, n_win, W)
        nodes = arr.transpose(1, 2, 0).reshape(n_win * W, OUT_SIZE)
        parts.append(nodes[:NPC])
    return np.concatenate(parts, axis=0).astype(np.float32)


def kernel(inp, src, dst, edge_val, weights, bias):
    nc, in_maps, n_win = _make_in_maps(inp, src, dst, edge_val, weights, bias)
    res = run_bass_kernel_spmd(nc, in_maps, list(range(N_CORES)))
    return _unshard(res, n_win)


# revision 19
# speedup vs baseline: 1.5616x; 1.1889x over previous
"""GCN layer (4-relation message passing) on 8 Trainium2 NeuronCores.

out = sum_r (A_r @ inp) @ W_r + sum_r b_r,  A_r in COO form (dst, src, val).

Sharding: edges sharded by dst range; core c owns dst in [c*12500, (c+1)*12500).
Edges are bucketed per (dst-window of W nodes, relation) cell and padded to
128-edge blocks. Host stages, per block, a [128, 64+W] bf16 slab: cols 0:64
hold the message rows inp[src], cols 64:64+W hold the selection matrix
S[p, j] = val_p * (j == dstloc_p)  (pure placement of input values - no
host arithmetic). Device does all FLOPs:

per (window w, relation r) cell, per block b:
  PE accumulates  aggT_wr [64f, W] += MSG_b^T @ S_b   in PSUM
  (edge_val scaling and the dst segment-sum happen inside this matmul).
Cells alternate between PSUM partition halves 0:64 / 64:128 so consecutive
cells' matmuls land in different PE column groups (weight-load overlap).
Phase 2 per window, relation-outer so the stationary W_r is loaded once per
group:  outT_w [64o, W] += matmul(lhsT=W_r[64f, 64o], rhs=aggT_wr)
plus bias via matmul(lhsT=bias[4, 64o], rhs=ones[4, W]).  PSUM->SBUF copies
alternate between the Scalar and Vector engines.
"""

import math
from contextlib import ExitStack

import numpy as np

import concourse.bass as bass
import concourse.tile as tile
from concourse import bacc, mybir
from concourse.bass_utils import run_bass_kernel_spmd

# problem constants
N_NODES = 100000
N_REL = 4
N_EDGES = 1600000
IN_SIZE = 64
OUT_SIZE = 64

N_CORES = 8
NPC = N_NODES // N_CORES  # nodes (dst) per core
P = 128                   # partitions / edges per block
W = 48                    # dst-window width (nodes per psum tile)
BW = IN_SIZE + W          # block slab width (msg cols + selection cols)
GW = 3                    # windows per slab DMA group

F32 = mybir.dt.float32
BF16 = mybir.dt.bfloat16


def _np_bf16():
    import ml_dtypes
    return ml_dtypes.bfloat16


def _host_prep(inp, src, dst, edge_val):
    """Bucket/pad edges per (core, window, rel); build block slabs."""
    n_win = math.ceil(NPC / W)
    ncell = n_win * N_REL
    srcf = src.reshape(-1).astype(np.int64)
    dstf = dst.reshape(-1).astype(np.int64)
    valf = edge_val.reshape(-1).astype(np.float32)
    rel = np.repeat(np.arange(N_REL, dtype=np.int64), src.shape[1])

    core = dstf // NPC
    dloc = dstf % NPC
    win = dloc // W
    wloc = dloc % W
    cell = win * N_REL + rel
    key = core * ncell + cell

    counts = np.bincount(key, minlength=N_CORES * ncell).reshape(
        N_CORES, ncell)
    B = np.maximum((counts.max(axis=0) + P - 1) // P, 1).astype(np.int64)
    starts = np.zeros(ncell + 1, dtype=np.int64)
    np.cumsum(B, out=starts[1:])
    T = int(starts[-1])

    edt = _np_bf16()
    slab = np.zeros((N_CORES, P, T, BW), dtype=edt)

    order = np.argsort(key, kind="stable")
    grp_start = np.zeros(N_CORES * ncell, dtype=np.int64)
    np.cumsum(counts.reshape(-1)[:-1], out=grp_start[1:])
    j = np.arange(len(order), dtype=np.int64) - grp_start[key[order]]
    t_col = starts[cell[order]] + (j // P)
    p_row = j % P
    c_ord = core[order]
    slab[c_ord, p_row, t_col, :IN_SIZE] = inp[srcf[order]].astype(edt)
    slab[c_ord, p_row, t_col, IN_SIZE + wloc[order]] = (
        valf[order].astype(edt))

    return n_win, B, starts, T, slab


_PROG_CACHE = {}


def _build_program(n_win, starts, T):
    key = (W, GW, tuple(int(s) for s in starts))
    if key in _PROG_CACHE:
        return _PROG_CACHE[key]

    nc = bacc.Bacc("TRN2", target_bir_lowering=False, debug=False,
                   num_devices=N_CORES)
    wcat = nc.dram_tensor("wcat", [IN_SIZE, N_REL * OUT_SIZE], BF16,
                          kind="ExternalInput").ap()
    biasc = nc.dram_tensor("biasc", [N_REL, OUT_SIZE], BF16,
                           kind="ExternalInput").ap()
    eslab = nc.dram_tensor("eslab", [P, T * BW], BF16,
                           kind="ExternalInput").ap()
    out = nc.dram_tensor("out", [OUT_SIZE, n_win * W], F32,
                         kind="ExternalOutput").ap()

    with tile.TileContext(nc) as tc, ExitStack() as ctx:
        p_const = ctx.enter_context(tc.tile_pool(name="p_const", bufs=1))
        p_msg = ctx.enter_context(tc.tile_pool(name="p_msg", bufs=3))
        p_agg = ctx.enter_context(
            tc.tile_pool(name="p_agg", bufs=2 * GW * N_REL + 2))
        p_out = ctx.enter_context(tc.tile_pool(name="p_out", bufs=1))
        ps_agg = ctx.enter_context(tc.tile_pool(name="ps_agg", bufs=3,
                                                space="PSUM"))
        ps_out = ctx.enter_context(tc.tile_pool(name="ps_out", bufs=GW,
                                                space="PSUM"))

        wt = p_const.tile([IN_SIZE, N_REL * OUT_SIZE], BF16)
        nc.sync.dma_start(wt[:], wcat[:])
        bt = p_const.tile([N_REL, OUT_SIZE], BF16)
        nc.sync.dma_start(bt[:], biasc[:])
        ones4 = p_const.tile([N_REL, W], BF16)
        nc.vector.memset(ones4[:], 1.0)
        outsb = p_out.tile([OUT_SIZE, n_win * W], F32)

        bg_max = max(
            int(starts[min(w0 + GW, n_win) * N_REL] - starts[w0 * N_REL])
            for w0 in range(0, n_win, GW))

        ncopy = 0
        for w0 in range(0, n_win, GW):
            w1 = min(w0 + GW, n_win)
            t0, t1 = int(starts[w0 * N_REL]), int(starts[w1 * N_REL])
            bg = t1 - t0
            mt = p_msg.tile([P, bg_max * BW], BF16, tag="msg")
            nc.sync.dma_start(mt[:, :bg * BW], eslab[:, t0 * BW:t1 * BW])

            aggs = {}
            for w in range(w0, w1):
                for r in range(N_REL):
                    c2 = w * N_REL + r
                    b0, b1 = int(starts[c2]) - t0, int(starts[c2 + 1]) - t0
                    ps = ps_agg.tile([IN_SIZE, W], F32, name="ps", tag="ps")
                    psh = ps[:]
                    for b in range(b0, b1):
                        o = b * BW
                        nc.tensor.matmul(
                            out=psh,
                            lhsT=mt[:, o:o + IN_SIZE],
                            rhs=mt[:, o + IN_SIZE:o + BW],
                            start=(b == b0), stop=(b == b1 - 1))
                    agg = p_agg.tile([IN_SIZE, W], BF16, name="agg",
                                     tag="agg")
                    if ncopy % 2 == 0:
                        nc.scalar.copy(agg[:], psh)
                    else:
                        nc.vector.tensor_copy(agg[:], psh)
                    ncopy += 1
                    aggs[(w, r)] = agg

            pos = {}
            for w in range(w0, w1):
                pos[w] = ps_out.tile([OUT_SIZE, W], F32, name="pos",
                                     tag="pos")
            for r in range(N_REL):
                for w in range(w0, w1):
                    nc.tensor.matmul(
                        out=pos[w][:],
                        lhsT=wt[:, r * OUT_SIZE:(r + 1) * OUT_SIZE],
                        rhs=aggs[(w, r)][:],
                        start=(r == 0), stop=False)
            for w in range(w0, w1):
                nc.tensor.matmul(
                    out=pos[w][:],
                    lhsT=bt[:],
                    rhs=ones4[:],
                    start=False, stop=True)
                if w % 2 == 0:
                    nc.scalar.copy(outsb[:, w * W:(w + 1) * W], pos[w][:])
                else:
                    nc.vector.tensor_copy(
                        outsb[:, w * W:(w + 1) * W], pos[w][:])

        nc.sync.dma_start(out[:], outsb[:])

    nc.compile()
    _PROG_CACHE[key] = nc
    return nc


def _make_in_maps(inp, src, dst, edge_val, weights, bias):
    inp = np.asarray(inp, dtype=np.float32)
    src = np.asarray(src)
    dst = np.asarray(dst)
    edge_val = np.asarray(edge_val, dtype=np.float32)
    weights = np.asarray(weights, dtype=np.float32)
    bias = np.asarray(bias, dtype=np.float32)

    n_win, B, starts, T, slab = _host_prep(inp, src, dst, edge_val)
    nc = _build_program(n_win, starts, T)

    edt = _np_bf16()
    wcat = np.ascontiguousarray(
        weights.transpose(1, 0, 2).reshape(IN_SIZE, N_REL * OUT_SIZE)
    ).astype(edt)
    biasc = bias.astype(edt)

    in_maps = []
    for c in range(N_CORES):
        in_maps.append({
            "wcat": wcat,
            "biasc": biasc,
            "eslab": slab[c].reshape(P, T * BW),
        })
    return nc, in_maps, n_win


def _unshard(res, n_win):
    parts = []
    for c in range(N_CORES):
        arr = res.results[c]["out"].reshape(OUT_SIZE, n_win, W)
        nodes = arr.transpose(1, 2, 0).reshape(n_win * W, OUT_SIZE)
        parts.append(nodes[:NPC])
    return np.concatenate(parts, axis=0).astype(np.float32)


def kernel(inp, src, dst, edge_val, weights, bias):
    nc, in_maps, n_win = _make_in_maps(inp, src, dst, edge_val, weights, bias)
    res = run_bass_kernel_spmd(nc, in_maps, list(range(N_CORES)))
    return _unshard(res, n_win)
